# revision 9
# baseline (speedup 1.0000x reference)
"""HQQ 1-bit quantized linear (out = x @ dequant(W).T + bias) on 8 Trainium2
NeuronCores.

Sharding: 8-way row-parallel. x rows (M=8192) split into 8 shards of 1024;
every core computes its [1024, 4096] output slab against the FULL weight
matrix (K=4096 contraction, O=4096 out features). This reads x from HBM
exactly once across the 8 cores (16 MiB/core vs 64 MiB/core for the 2x4
tensor-parallel split), so DMA stays far below the PE roofline.

Per core the device kernel:
  - keeps the x shard resident in SBUF as bf16 [128, 32 kt, 1024] (cast from
    the f32 DMA),
  - streams the packed weights + per-group affine coefficients per o-chunk of
    512 out-features, dequantizing on DVE (shift/and bit-extract, then
    B*s + (-z*s)) into a double-buffered bf16 W tile [128, 32 kt, 512],
  - runs the 32-k-tile accumulation per (m-tile, o-chunk) on the tensor
    engine into fp32 PSUM (8 banks round-robin),
  - drains PSUM via DVE with a fused bias add (bias pre-broadcast across
    partitions once at setup via a rank-1 ones x bias matmul).

Host-side work is layout/packing only: transpose/permute/replicate/slice,
int16 container cast for the packed bytes, bf16 cast + per-group (-z*s)
product for the tiny [4096, 64] coefficient arrays.
"""

import sys

for _p in ("/opt/trn_rl_repo", "/root/.axon_site/_ro/trn_rl_repo"):
    if _p not in sys.path:
        sys.path.append(_p)

import numpy as np

P = 128
OC = 512                      # out-feature chunk per dequant/matmul round
NBITS_PER_BYTE = 8
GROUP_SIZE = 64
M_FULL, K_IN, O_FULL = 8192, 4096, 4096
N_CORES = 8
M_SH = M_FULL // N_CORES      # 1024 rows per core

_compiled = {}


def _build_nc():
    import concourse.bacc as bacc
    import concourse.mybir as mybir
    import concourse.tile as tile

    f32 = mybir.dt.float32
    bf16 = mybir.dt.bfloat16
    i16 = mybir.dt.int16

    PB = K_IN // NBITS_PER_BYTE   # 512 packed-byte rows
    N_KT = K_IN // P              # 32 k-tiles
    N_V = PB // P                 # 4 byte-tiles
    N_MT = M_SH // P              # 8 m-tiles
    N_OC = O_FULL // OC           # 8 o-chunks

    nc = bacc.Bacc("TRN2", target_bir_lowering=False, debug=False,
                   num_devices=N_CORES)

    xt_d = nc.dram_tensor("xt", [K_IN, M_SH], f32, kind="ExternalInput")
    wpt_d = nc.dram_tensor("wpt", [PB, O_FULL], i16, kind="ExternalInput")
    sexp_d = nc.dram_tensor("sexp", [PB, O_FULL], bf16, kind="ExternalInput")
    nzs_d = nc.dram_tensor("nzs", [PB, O_FULL], bf16, kind="ExternalInput")
    bias_d = nc.dram_tensor("bias", [P, O_FULL], bf16, kind="ExternalInput")
    out_d = nc.dram_tensor("out", [M_SH, O_FULL], f32, kind="ExternalOutput")

    with tile.TileContext(nc) as tc:
        with tc.tile_pool(name="fixed", bufs=1) as fixed, \
             tc.tile_pool(name="xtf", bufs=2) as xtf_pool, \
             tc.tile_pool(name="wload", bufs=2) as wload_pool, \
             tc.tile_pool(name="deq", bufs=1) as deq_pool, \
             tc.tile_pool(name="wt", bufs=2) as wt_pool, \
             tc.tile_pool(name="outp", bufs=2) as out_pool, \
             tc.tile_pool(name="psum", bufs=8, space="PSUM") as psum_pool:

            # ---- bias pre-broadcast on host: [128, O_FULL] bf16 ----
            bias_bc = fixed.tile([P, O_FULL], bf16, tag="biasbc")
            nc.sync.dma_start(bias_bc[:], bias_d[:, :])

            # ---- resident x shard: bf16 [128, N_KT, M_SH] (cast on ScalarE,
            # keeping DVE free for dequant) ----
            xb = fixed.tile([P, N_KT, M_SH], bf16, tag="xb")
            for mi in range(N_MT):
                xt_f = xtf_pool.tile([P, N_KT, P], f32, tag="xtf", name="xt_f")
                nc.sync.dma_start(
                    xt_f[:],
                    xt_d[:, mi * P:(mi + 1) * P].rearrange("(t p) m -> p t m", p=P))
                nc.scalar.copy(xb[:, :, mi * P:(mi + 1) * P], xt_f[:])

            # ---- o-chunk loop: stream-dequant W chunk, then matmul ----
            for oc in range(N_OC):
                osl = slice(oc * OC, (oc + 1) * OC)
                wpt_t = wload_pool.tile([P, N_V, OC], i16, tag="wpt", name="wpt_t")
                s_t = wload_pool.tile([P, N_V, OC], bf16, tag="s", name="s_t")
                nzs_t = wload_pool.tile([P, N_V, OC], bf16, tag="nzs", name="nzs_t")
                nc.sync.dma_start(
                    wpt_t[:], wpt_d[:, osl].rearrange("(v p) o -> p v o", p=P))
                nc.sync.dma_start(
                    s_t[:], sexp_d[:, osl].rearrange("(v p) o -> p v o", p=P))
                nc.sync.dma_start(
                    nzs_t[:], nzs_d[:, osl].rearrange("(v p) o -> p v o", p=P))

                # dequant batched per bit-plane u: k-tiles t = 4u + v, v=0..3
                # are contiguous in WT, so one [128, 4, OC] op covers all 4.
                # Bit-extract + scale on DVE, the affine add on GpSimd.
                WT = wt_pool.tile([P, N_KT, OC], bf16, tag="WT", name="WT")
                for u in range(N_KT // N_V):
                    B_u = deq_pool.tile([P, N_V, OC], i16, tag="B", name="B_u")
                    nc.vector.tensor_scalar(
                        B_u[:], wpt_t[:], u, 1,
                        mybir.AluOpType.logical_shift_right,
                        mybir.AluOpType.bitwise_and)
                    bs_u = deq_pool.tile([P, N_V, OC], bf16, tag="bs", name="bs_u")
                    nc.vector.scalar_tensor_tensor(
                        bs_u[:], B_u[:], 1.0, s_t[:],
                        mybir.AluOpType.mult, mybir.AluOpType.mult)
                    nc.gpsimd.tensor_tensor(
                        WT[:, u * N_V:(u + 1) * N_V, :], bs_u[:], nzs_t[:],
                        mybir.AluOpType.add)

                for mi in range(N_MT):
                    ps = psum_pool.tile([P, OC], f32, tag="ps", name="ps")
                    for t in range(N_KT):
                        nc.tensor.matmul(
                            ps[:], xb[:, t, mi * P:(mi + 1) * P], WT[:, t, :],
                            start=(t == 0), stop=(t == N_KT - 1))
                    out_t = out_pool.tile([P, OC], f32, tag="out", name="out_t")
                    nc.vector.tensor_tensor(out_t[:], ps[:], bias_bc[:, osl],
                                            mybir.AluOpType.add)
                    nc.sync.dma_start(out_d[mi * P:(mi + 1) * P, osl], out_t[:])
    nc.compile()
    return nc


def _get_nc(**kw):
    key = tuple(sorted(kw.items()))
    if key not in _compiled:
        _compiled[key] = _build_nc(**kw)
    return _compiled[key]


def _host_prep(x, W_packed, scale, zero, bias):
    """Layout/packing-only prep of per-core input maps."""
    import ml_dtypes
    bf16 = ml_dtypes.bfloat16
    PB = K_IN // NBITS_PER_BYTE
    NG = K_IN // GROUP_SIZE
    x = np.asarray(x, dtype=np.float32)
    W_packed = np.asarray(W_packed)
    scale2d = np.asarray(scale, dtype=np.float32).reshape(O_FULL, NG)
    zero2d = np.asarray(zero, dtype=np.float32).reshape(O_FULL, NG)
    bias = np.asarray(bias, dtype=np.float32)

    # shared (replicated) tensors
    wpt = np.ascontiguousarray(W_packed.T.astype(np.int16))          # [PB, O]
    sexp = np.ascontiguousarray(
        np.repeat(scale2d.T, NBITS_PER_BYTE, axis=0).astype(bf16))   # [PB, O]
    nzs = np.ascontiguousarray(
        np.repeat((-zero2d * scale2d).T, NBITS_PER_BYTE, axis=0).astype(bf16))
    bias2 = np.ascontiguousarray(
        np.broadcast_to(bias.astype(bf16)[None, :], (P, O_FULL)))

    in_maps = []
    for c in range(N_CORES):
        xs = x[c * M_SH:(c + 1) * M_SH]                              # [M_SH, K]
        # bit-plane-major permuted transpose: xt[u*PB + pb, m] = x[m, 8*pb + u]
        xt = xs.T.reshape(PB, NBITS_PER_BYTE, M_SH)
        xt = np.ascontiguousarray(xt.transpose(1, 0, 2).reshape(K_IN, M_SH))
        in_maps.append(dict(xt=xt, wpt=wpt, sexp=sexp, nzs=nzs, bias=bias2))
    return in_maps


def run_sharded(x, W_packed, scale, zero, bias, trace=False, **run_kwargs):
    """Compile (cached), run on 8 cores, return (full_out, BassKernelResults)."""
    from concourse.bass_utils import run_bass_kernel_spmd

    nc = _get_nc()
    in_maps = _host_prep(x, W_packed, scale, zero, bias)
    res = run_bass_kernel_spmd(nc, in_maps, core_ids=list(range(N_CORES)),
                               trace=trace, **run_kwargs)
    out = np.empty((M_FULL, O_FULL), dtype=np.float32)
    for c in range(N_CORES):
        out[c * M_SH:(c + 1) * M_SH, :] = res.results[c]["out"]
    return out, res


def kernel(x, W_packed, scale, zero, bias):
    out, _ = run_sharded(x, W_packed, scale, zero, bias)
    return out
